# revision 28
# baseline (speedup 1.0000x reference)
"""Trainium2 Bass kernel for nn_EstimatorQNN (18-qubit QNN, batch 16).

Math: the circuit is RX-encoding (product state) + 3 layers of
(RY on every wire, CZ chain). All CZs are diagonal, so in the Heisenberg
picture Z_i only spreads to wires {i-2..i+2}: D3/R3 keep it on wire i,
D2 and D1 each grow support by one wire per side, and every
boundary-crossing CZ commutes with the operator at its application time.
Hence each <Z_i> equals an exact 5-qubit (32-amplitude) simulation over
the window {i-2..i+2} (out-of-range slots padded with angle-0 gates).
Additional exact cuts: layer-3 CZs don't change probabilities (|±a|^2),
and layer-3 RYs on wires != i commute with Z_i — both dropped.

After RX encoding the state is m(f) * (-i)^popcount(f) with real m, and
all remaining gates are real, so re/im parts evolve independently as two
real vectors.

Layout per core: 36 sims (2 samples x 18 windows) on partitions; the
64-wide free axis interleaves (amplitude f, part t) at col 2*f+t, which
keeps every amplitude-bit slice a 2-free-dim access pattern. RY on
window-slot k pairs amplitudes along free-dim bit k via strided APs;
angles are per-partition scalars. 8 cores shard the batch (2 samples
each).

Implementation: raw Bass blocks (no Tile — this walrus build only
encodes one semaphore wait per instruction, which Tile's multi-wait
drain violates), one serial DVE chain. sin/cos are evaluated as DVE
polynomials (deg-7/deg-8 minimax on |x|<=1.8, abs err < 4e-6) — this
avoids the ScalarEngine entirely, including the ~1.3us Sin ACT-table
load and two cross-engine hops.

DVE chaining hazard (probed on HW): a dependent op immediately after
its producer reads stale data unless its scalar operands are
per-partition SBUF APs (the scalar fetch delays the stream enough);
ops with immediate scalars, InstTensorTensor (tensor_mul/add), and
tensor_copy all mis-read a just-written tile. Hence every op below is
tensor_scalar / scalar_tensor_tensor with scalars taken from DMA'd
constant columns, which is deterministic-correct across repeated runs.
"""

import sys

sys.path.insert(0, "/opt/trn_rl_repo")

import numpy as np

import concourse.bass as bass
import concourse.mybir as mybir
from concourse.bass_utils import run_bass_kernel_spmd

NQ = 18
BATCH = 16
NCORES = 8
SPB = BATCH // NCORES  # samples per core
ROWS = SPB * NQ  # 36 sims per core
W = 5  # window width
NA = 32  # amplitudes per window sim
NANG = 32  # angle cols 0-15 used (5 x-window, 5 L1, 5 L2, 1 L3-center);
# cols 16-31 zero padding (keeps every trig op at 128B/partition).
NK = 12  # const-scalar cols: S0-S3, C0-C4, 0.5, 1.0, pad
# input cols: [angles(32) | consts(12) | init_phase(64) | cz(64) | zsign(64)]
C_ANG = 0
C_K = NANG
C_ST = C_K + NK
C_CZ = C_ST + 2 * NA
C_ZS = C_CZ + 2 * NA
CC = C_ZS + 2 * NA  # 236

F32 = mybir.dt.float32
ALU = mybir.AluOpType

# sin(x)/x = S0 + S1 x^2 + S2 x^4 + S3 x^6 ; cos(x) = C0 + ... + C4 x^8
# (Chebyshev-weighted LS fit on [-1.8, 1.8]; f32 abs err < 4e-6)
S0, S1, S2, S3 = (0.9999967030351217, -0.16663804531081605,
                  0.00829328200083101, -0.0001800097882873528)
C0, C1, C2, C3, C4 = (0.9999997361888308, -0.49999632529198995,
                      0.041658313179372915, -0.0013821368414218083,
                      2.2536401738981138e-05)


def _const_block() -> np.ndarray:
    """[ROWS, 192] constant cols: init phases, cz mask, zsign mask.

    col 2*f+t holds amplitude f, part t (t=0 re, t=1 im).
    """
    f = np.arange(NA)
    bits = (f[:, None] >> np.arange(W)[None, :]) & 1  # [32, 5]
    pop = bits.sum(1)
    re_ph = np.array([1.0, 0.0, -1.0, 0.0])[pop % 4]
    im_ph = np.array([0.0, -1.0, 0.0, 1.0])[pop % 4]
    init = np.stack([re_ph, im_ph], axis=1).reshape(-1)  # interleaved
    ncz = sum(bits[:, k] & bits[:, k + 1] for k in range(W - 1))
    cz = np.repeat((-1.0) ** ncz, 2)
    zs = np.repeat(1.0 - 2.0 * bits[:, 2], 2)
    ks = [S0, S1, S2, S3, C0, C1, C2, C3, C4, 0.5, 1.0, 0.0]
    const = np.concatenate([ks, init, cz, zs]).astype(np.float32)
    return np.broadcast_to(const, (ROWS, const.size)).copy()


def _angle_table(x: np.ndarray, params: np.ndarray) -> np.ndarray:
    """[BATCH, NQ, NANG] per-sim angles (0 for padded window slots)."""
    w = params[NQ:]
    A = np.zeros((BATCH, NQ, NANG), np.float32)  # cols 16+ stay zero
    for i in range(NQ):
        for k in range(W):
            j = i - 2 + k
            if 0 <= j < NQ:
                A[:, i, k] = x[:, j]
                A[:, i, W + k] = w[j]
                A[:, i, 2 * W + k] = w[NQ + j]
        A[:, i, 3 * W] = w[2 * NQ + i]
    return A


def _bitview(ap64, k: int, b: int):
    """View of a [ROWS, 64] re/im-interleaved AP selecting amplitude-bit
    k == b (both re and im). 2 free dims: [2^(4-k), 2^(k+1)]."""
    h = NA >> (k + 1)
    m = 2 << k
    v = ap64.rearrange("p (h c m) -> p h c m", h=h, c=2, m=m)
    return v[:, :, b, :]


def _build_nc(detect_races: bool = True) -> bass.Bass:
    # detect_races=False for CoreSim runs: the race detector doesn't model
    # the DVE per-op DRAIN that orders the in-place chain on hardware.
    nc = bass.Bass(detect_race_conditions=detect_races)
    inp = nc.dram_tensor("inp", [ROWS, CC], F32, kind="ExternalInput")
    outp = nc.dram_tensor("outp", [ROWS, 1], F32, kind="ExternalOutput")

    with (
        nc.sbuf_tensor([128, CC], F32) as IN,
        nc.sbuf_tensor([128, NANG], F32) as CS,
        nc.sbuf_tensor([128, NANG], F32) as SN,
        nc.sbuf_tensor([128, NANG], F32) as HH,
        nc.sbuf_tensor([128, NANG], F32) as X2,
        nc.sbuf_tensor([128, NANG], F32) as X4,
        nc.sbuf_tensor([128, NANG], F32) as TA,
        nc.sbuf_tensor([128, NANG], F32) as TB,
        nc.sbuf_tensor([128, 2 * NA], F32) as T,
        nc.sbuf_tensor([128, 2 * NA], F32) as SCR,
        nc.sbuf_tensor([128, 2], F32) as RES,
        nc.semaphore() as dma_sem,
        nc.semaphore() as dve_sem,
        nc.Block() as block,
    ):
        ang = IN[0:ROWS, C_ANG:C_ANG + NANG]

        def K(i):  # per-partition const-scalar column
            return IN[0:ROWS, C_K + i:C_K + i + 1]

        (k_s0, k_s1, k_s2, k_s3, k_c0, k_c1, k_c2, k_c3, k_c4,
         k_half, k_one) = [K(i) for i in range(11)]
        state = IN[0:ROWS, C_ST:C_ST + 2 * NA]
        czm = IN[0:ROWS, C_CZ:C_CZ + 2 * NA]
        zsm = IN[0:ROWS, C_ZS:C_ZS + 2 * NA]
        cs = CS[0:ROWS, :]
        sn = SN[0:ROWS, :]
        hh = HH[0:ROWS, :]
        x2 = X2[0:ROWS, :]
        x4 = X4[0:ROWS, :]
        ta = TA[0:ROWS, :]
        tb = TB[0:ROWS, :]
        t64 = T[0:ROWS, :]
        scr = SCR[0:ROWS, :]
        res = RES[0:ROWS, 0:1]

        @block.sync
        def _(sync):
            sync.dma_start(out=IN[0:ROWS, :], in_=inp[:, :]).then_inc(
                dma_sem, 16)
            sync.wait_ge(dve_sem, 1)
            sync.dma_start(out=outp[:, :], in_=res).then_inc(dma_sem, 16)

        @block.vector
        def _(vector):
            vector.wait_ge(dma_sem, 16)

            # --- trig: x = ang/2; sn = sin(x), cs = cos(x) ---
            # HAZARD RULE (probed on HW): dependent DVE ops chain safely
            # only when their scalar operands are per-partition APs; ops
            # with immediate scalars (and InstTensorTensor / tensor_copy)
            # read stale data from a just-written producer. All scalars
            # below are DMA'd const columns.
            stt = vector.scalar_tensor_tensor
            ts = vector.tensor_scalar
            vector.tensor_scalar_mul(hh, ang, k_half)
            stt(x2, hh, k_one, hh, ALU.mult, ALU.mult)
            stt(x4, x2, k_one, x2, ALU.mult, ALU.mult)
            # sin = x * ((S0 + S1 x2) + x4*(S2 + S3 x2))
            ts(ta, x2, k_s1, k_s0, ALU.mult, ALU.add)
            ts(tb, x2, k_s3, k_s2, ALU.mult, ALU.add)
            stt(tb, tb, k_one, x4, ALU.mult, ALU.mult)
            stt(ta, ta, k_one, tb, ALU.mult, ALU.add)
            stt(sn, ta, k_one, hh, ALU.mult, ALU.mult)
            # cos = (C0 + C1 x2) + x4*((C2 + C3 x2) + C4 x4)
            ts(ta, x2, k_c1, k_c0, ALU.mult, ALU.add)
            ts(tb, x2, k_c3, k_c2, ALU.mult, ALU.add)
            stt(tb, x4, k_c4, tb, ALU.mult, ALU.add)
            stt(tb, tb, k_one, x4, ALU.mult, ALU.mult)
            stt(cs, ta, k_one, tb, ALU.mult, ALU.add)

            # --- init: state starts as phase masks; fold in per-slot c/s
            for k in range(W):
                s0 = _bitview(state, k, 0)
                s1 = _bitview(state, k, 1)
                vector.tensor_scalar_mul(s0, s0, cs[:, k:k + 1])
                vector.tensor_scalar_mul(s1, s1, sn[:, k:k + 1])

            def ry(k: int, col: int):
                c = cs[:, col:col + 1]
                s = sn[:, col:col + 1]
                # T = s * state (all amplitudes, both parts)
                vector.tensor_scalar_mul(t64, state, s)
                a0 = _bitview(state, k, 0)
                a1 = _bitview(state, k, 1)
                t0 = _bitview(t64, k, 0)
                t1 = _bitview(t64, k, 1)
                # a0' = c*a0 - s*a1 ; a1' = c*a1 + s*a0
                vector.scalar_tensor_tensor(
                    a0, a0, c, t1, ALU.mult, ALU.subtract)
                vector.scalar_tensor_tensor(
                    a1, a1, c, t0, ALU.mult, ALU.add)

            for k in range(W):  # layer 1
                ry(k, W + k)
            stt(state, state, k_one, czm, ALU.mult, ALU.mult)
            for k in range(W):  # layer 2
                ry(k, 2 * W + k)
            stt(state, state, k_one, czm, ALU.mult, ALU.mult)
            ry(2, 3 * W)  # layer 3: only the center RY affects <Z_center>

            # <Z> = sum_f (re^2 + im^2) * zsign
            stt(t64, state, k_one, zsm, ALU.mult, ALU.mult)
            stt(
                scr, state, k_one, t64, ALU.mult, ALU.mult, accum_out=res,
            ).then_inc(dve_sem, 1)

    return nc


_NC_CACHE = None


def _get_nc():
    global _NC_CACHE
    if _NC_CACHE is None:
        _NC_CACHE = _build_nc()
    return _NC_CACHE


def _in_maps(x, params):
    A = _angle_table(x, params)  # [BATCH, NQ, NANG]
    const = _const_block()  # [ROWS, 192]
    maps = []
    for c in range(NCORES):
        ang = A[c * SPB:(c + 1) * SPB].reshape(ROWS, NANG)
        maps.append(
            {"inp": np.ascontiguousarray(
                np.concatenate([ang, const], axis=1), np.float32)}
        )
    return maps


def _run(x, params, trace=False):
    x = np.ascontiguousarray(np.asarray(x, np.float32))
    params = np.ascontiguousarray(np.asarray(params, np.float32))
    res = run_bass_kernel_spmd(
        _get_nc(), _in_maps(x, params), list(range(NCORES)), trace=trace)
    out = np.concatenate(
        [res.results[c]["outp"].reshape(SPB, NQ) for c in range(NCORES)],
        axis=0,
    ).astype(np.float32)
    return out, res


def kernel(x, params):
    out, _ = _run(x, params)
    return out
